# revision 15
# baseline (speedup 1.0000x reference)
"""Trainium2 Bass kernel for nn_DFlashDecoderLayer (dense transformer decoder layer:
self-attn + cross-attn + SwiGLU MLP, B=1, S=2048, H=1024, NH=16, HD=64, I=4096).

Sharding strategy (8 NeuronCores, SPMD):
  Sequence-sharded: core r owns query rows [256r, 256r+256).  Every matmul weight is
  used in full by every core, in bf16.  The only cross-core data dependency is full-
  sequence K/V for the two attention blocks; each core computes K/V for its own rows
  (all heads) and a single AllGather per attention block shares them (1MB/rank).
  Norm weights + 1/sqrt(HD) score scale are folded into the following projection
  weights host-side.  Residual path stays fp32; matmuls run bf16 with fp32 PSUM
  accumulation; softmax exp in fp32->bf16 without max-subtraction (|scores| < ~5).

  On-chip layout is feature-major ("transposed"): activations live as [H_part, seq]
  so weight matrices ([in, out] row-major) serve directly as matmul lhsT tiles and
  layer outputs chain without transposes.  Scores are computed transposed [k, q];
  the softmax denominator comes from a ones-column augmented onto V (M=65 matmul),
  and the 1/l division is applied via gpsimd partition-broadcast before o_proj.
"""

import os
import sys

sys.path.insert(0, "/opt/trn_rl_repo")

import numpy as np
import ml_dtypes

import concourse.bass as bass
import concourse.mybir as mybir
import concourse.tile as tile

H = 1024      # hidden size
S = 2048      # sequence length
NH = 16       # heads
HD = 64       # head dim
I = 4096      # mlp intermediate
NC = 8        # cores
R = S // NC   # rows per core = 256
HT = H // 128  # hidden tiles = 8
KT = S // 128  # key tiles = 16
EPS = 1e-6

F32 = mybir.dt.float32
BF16 = mybir.dt.bfloat16
AF = mybir.ActivationFunctionType
BF16NP = ml_dtypes.bfloat16

_CACHED_MODULE = None


def _split_multi_waits(nc):
    """This env's walrus rejects >1 sem wait per instruction.
    Hoist extra waits onto preceding single-wait NoOps on the same engine."""
    limit = 1
    n_split = 0
    for f in nc.m.functions:
        for bb in f.blocks:
            new_insts = []
            for inst in bb.instructions:
                si = getattr(inst, "sync_info", None)
                if (si is not None and getattr(inst, "engine", None) is not None
                        and len(si.on_wait) > limit):
                    waits = list(si.on_wait)
                    hoist, keep = waits[:-limit], waits[-limit:]
                    for i, w in enumerate(hoist):
                        new_insts.append(
                            mybir.InstNoOp(
                                name=f"{inst.name}_waitsplit_{i}",
                                engine=inst.engine,
                                sync_info=mybir.SyncInfo(on_wait=[w], on_update=[]),
                                bass_nofuse=True,
                            )
                        )
                        n_split += 1
                    si.on_wait = keep
                new_insts.append(inst)
            bb.instructions = new_insts
    return n_split


def build_module():
    global _CACHED_MODULE
    if _CACHED_MODULE is not None:
        return _CACHED_MODULE

    nc = bass.Bass(num_devices=NC)

    # --- kernel I/O (per-core) ---
    xT = nc.declare_dram_parameter("xT", [H, R], F32, isOutput=False)
    ctxT = nc.declare_dram_parameter("ctxT", [H, R], BF16, isOutput=False)
    wnames = ["sa_wq", "sa_wk", "sa_wv", "sa_wo", "ca_wq", "ca_wk", "ca_wv", "ca_wo"]
    W = {n: nc.declare_dram_parameter(n, [H, H], BF16, isOutput=False) for n in wnames}
    W["w_gate"] = nc.declare_dram_parameter("w_gate", [H, I], BF16, isOutput=False)
    W["w_up"] = nc.declare_dram_parameter("w_up", [H, I], BF16, isOutput=False)
    W["w_down"] = nc.declare_dram_parameter("w_down", [I, H], BF16, isOutput=False)
    outT = nc.declare_dram_parameter("outT", [H, R], F32, isOutput=True)

    groups = [list(range(NC))]

    with tile.TileContext(nc) as tc:
        with (
            tc.tile_pool(name="p1", bufs=1) as p1,        # long-lived singles
            tc.tile_pool(name="p2", bufs=2) as p2,        # rotating pairs
            tc.tile_pool(name="resid", bufs=2) as presid, # xT / h1 / h2 fp32
            tc.tile_pool(name="wts", bufs=4) as pw,       # 2MB weight slabs
            tc.tile_pool(name="psA", bufs=4, space="PSUM") as psA,
            tc.tile_pool(name="psB", bufs=4, space="PSUM") as psB,
            tc.tile_pool(name="dram", bufs=1, space="DRAM") as pdram,
        ):
            # --- constants ---
            inv_h = p1.tile([128, 1], F32, tag="inv_h")
            nc.vector.memset(inv_h[:], 1.0 / H)
            eps_c = p1.tile([1, 1], F32, tag="eps_c")
            nc.vector.memset(eps_c[:], EPS)
            # ones row for K=1 outer-product broadcasts (partition_broadcast is
            # unsupported by this walrus build)
            ones_row = p1.tile([1, 128], F32, tag="ones_row")
            nc.vector.memset(ones_row[:], 1.0)

            # --- input loads ---
            xt_sb = presid.tile([128, HT, R], F32, tag="resid", name="xt_sb")
            nc.sync.dma_start(xt_sb[:], xT.rearrange("(t p) q -> p t q", p=128))
            ctx_sb = p1.tile([128, HT, R], BF16, tag="ctx_sb")
            nc.sync.dma_start(ctx_sb[:], ctxT.rearrange("(t p) q -> p t q", p=128))

            def load_w(dram_t, cols=None, rows=None, name="w"):
                """Load a [1024, M<=1024] slab of a weight matrix as [128, 8, M] bf16."""
                ap = dram_t.rearrange("(t p) m -> p t m", p=128)
                if rows is not None:  # row-chunk of a tall matrix (w_down)
                    ap = dram_t[rows[0]:rows[1], :].rearrange("(t p) m -> p t m", p=128)
                if cols is not None:
                    ap = ap[:, :, cols[0]:cols[1]]
                m = ap.shape[2]
                t = pw.tile([128, HT, 1024], BF16, tag="w", name=name)
                nc.sync.dma_start(t[:, :, :m], ap)
                return t

            def rmsnorm(src_f32, dst_name):
                """src [128, HT, R] f32 -> normalized bf16 [128, HT, R] (no weight)."""
                var = psA.tile([128, 512], F32, tag="psA", name=f"{dst_name}_var")
                for t in range(HT):
                    sq = p2.tile([128, R], F32, tag="sq", name=f"{dst_name}_sq{t}")
                    nc.scalar.activation(sq[:], src_f32[:, t, :], AF.Square)
                    nc.tensor.matmul(var[:1, :R], inv_h[:], sq[:],
                                     start=(t == 0), stop=(t == HT - 1))
                sd = p2.tile([1, R], F32, tag="sd", name=f"{dst_name}_sd")
                nc.scalar.activation(sd[:], var[:1, :R], AF.Sqrt, bias=eps_c[:])
                rstd = p2.tile([1, R], F32, tag="rstd", name=f"{dst_name}_rstd")
                nc.vector.reciprocal(rstd[:], sd[:])
                rb = psA.tile([128, 512], F32, tag="psA", name=f"{dst_name}_rb")
                nc.tensor.matmul(rb[:, :R], ones_row[:1, :], rstd[:1, :],
                                 start=True, stop=True)
                dst = p1.tile([128, HT, R], BF16, tag="normed", name=dst_name)
                for t in range(HT):
                    nc.vector.tensor_mul(dst[:, t, :], src_f32[:, t, :], rb[:, :R])
                return dst

            def proj(w_sb, act_sb, dst_bf16):
                """dst[*, m, :] (bf16 [128, HT, R]) = (W.T @ act), W slab [128, HT, 1024]."""
                for m in range(HT):
                    ps = psA.tile([128, 512], F32, tag="psA", name=f"pj_{m}")
                    for t in range(HT):
                        nc.tensor.matmul(ps[:, :R], w_sb[:, t, 128 * m:128 * (m + 1)],
                                         act_sb[:, t, :], start=(t == 0), stop=(t == HT - 1))
                    nc.vector.tensor_copy(dst_bf16[:, m, :], ps[:, :R])

            def proj_add(w_sb, act_sb, resid_f32, dst_f32):
                """dst (f32 [128, HT, R]) = resid + W.T @ act."""
                for m in range(HT):
                    ps = psA.tile([128, 512], F32, tag="psA", name=f"pa_{m}")
                    for t in range(HT):
                        nc.tensor.matmul(ps[:, :R], w_sb[:, t, 128 * m:128 * (m + 1)],
                                         act_sb[:, t, :], start=(t == 0), stop=(t == HT - 1))
                    nc.vector.tensor_add(dst_f32[:, m, :], ps[:, :R], resid_f32[:, m, :])

            def kv_block(wk_sb, wv_sb, act_sb, blk):
                """Compute own-row K^T [1024, R] and V [R, 1024] (bf16), AllGather
                each across cores (K first — scores only need K, so attention can
                start while the V gather is still in flight)."""
                k_in = pdram.tile([H * R], BF16, tag=f"kin{blk}")
                k_out = pdram.tile([NC, H * R], BF16, tag=f"kout{blk}",
                                   addr_space="Shared")
                v_in = pdram.tile([H * R], BF16, tag=f"vin{blk}")
                v_out = pdram.tile([NC, H * R], BF16, tag=f"vout{blk}",
                                   addr_space="Shared")
                k_view = k_in.rearrange("(t p q) -> p t q", t=HT, p=128, q=R)
                v_view = v_in.rearrange("(mt p d) -> p mt d", mt=2, p=128, d=1024)
                for m in range(HT):
                    ps = psA.tile([128, 512], F32, tag="psA", name=f"k{blk}_{m}")
                    for t in range(HT):
                        nc.tensor.matmul(ps[:, :R], wk_sb[:, t, 128 * m:128 * (m + 1)],
                                         act_sb[:, t, :], start=(t == 0), stop=(t == HT - 1))
                    stg = p2.tile([128, 512], BF16, tag="stg", bufs=3, name=f"ks{blk}_{m}")
                    nc.vector.tensor_copy(stg[:, :R], ps[:, :R])
                    nc.sync.dma_start(k_view[:, m, :], stg[:, :R])
                nc.gpsimd.collective_compute(
                    "AllGather", mybir.AluOpType.bypass, replica_groups=groups,
                    ins=[k_in[:]], outs=[k_out[:]])
                for mt in range(2):
                    for nchunk in range(2):
                        ps = psA.tile([128, 512], F32, tag="psA", name=f"v{blk}_{mt}_{nchunk}")
                        for t in range(HT):
                            nc.tensor.matmul(
                                ps[:], act_sb[:, t, 128 * mt:128 * (mt + 1)],
                                wv_sb[:, t, 512 * nchunk:512 * (nchunk + 1)],
                                start=(t == 0), stop=(t == HT - 1))
                        stg = p2.tile([128, 512], BF16, tag="stg", bufs=3,
                                      name=f"vs{blk}_{mt}_{nchunk}")
                        nc.vector.tensor_copy(stg[:], ps[:])
                        nc.sync.dma_start(v_view[:, mt, 512 * nchunk:512 * (nchunk + 1)], stg[:])
                nc.gpsimd.collective_compute(
                    "AllGather", mybir.AluOpType.bypass, replica_groups=groups,
                    ins=[v_in[:]], outs=[v_out[:]])
                return k_out, v_out

            def attention(q_sb, k_out, v_out, blk):
                """q_sb [128, HT, R] bf16 (feature-major, all heads), k_out/v_out from
                kv_block. Returns attnT [128, HT, R] bf16 = softmax(qk)V transposed."""
                vsb = p1.tile([128, KT, NH, HD + 1], BF16, tag="vsb", name=f"vsb{blk}")
                attnT = p1.tile([128, HT, R], BF16, tag="attnT", name=f"attnT{blk}")
                for dt in range(HT):  # head pair dt = heads 2dt, 2dt+1
                    kf = p2.tile([128, NC, R], BF16, tag="kf", name=f"kf{blk}_{dt}")
                    for r in range(NC):
                        nc.sync.dma_start(
                            kf[:, r, :],
                            k_out[r].rearrange("(t p q) -> p t q", t=HT, p=128, q=R)[:, dt, :])
                    if dt == 0:
                        # V table loads queue AFTER the first K loads so head-0
                        # scores aren't stuck behind 4MB of V DMA.
                        for kt in range(KT):
                            r, mt = kt // 2, kt % 2
                            src = v_out[r].rearrange(
                                "(mt p hd d) -> p mt hd d", mt=2, p=128, hd=NH, d=HD)
                            nc.sync.dma_start(vsb[:, kt, :, 0:HD], src[:, mt, :, :])
                        nc.vector.memset(vsb[:, :, :, HD:HD + 1], 1.0)
                    for hh in range(2):
                        h = 2 * dt + hh
                        off = HD * hh
                        pt = p2.tile([128, KT, R], BF16, tag="pt", bufs=3, name=f"pt{blk}_{h}")
                        for kp in range(KT // 2):
                            sps = psA.tile([128, 512], F32, tag="psA", name=f"s{blk}_{h}_{kp}")
                            for j in range(2):
                                kt = 2 * kp + j
                                r2, mt2 = kt // 2, kt % 2
                                nc.tensor.matmul(
                                    sps[:, R * j:R * (j + 1)],
                                    kf[off:off + HD, r2, 128 * mt2:128 * (mt2 + 1)],
                                    q_sb[off:off + HD, dt, :],
                                    start=True, stop=True, tile_position=(off, 0))
                            nc.scalar.activation(
                                pt[:, 2 * kp:2 * kp + 2, :],
                                sps.rearrange("p (a b) -> p a b", a=2), AF.Exp)
                        avps = psB.tile([128, 512], F32, tag="psB", name=f"av{blk}_{h}")
                        for kt in range(KT):
                            nc.tensor.matmul(avps[:HD + 1, :R], vsb[:, kt, h, :],
                                             pt[:, kt, :], start=(kt == 0), stop=(kt == KT - 1))
                        rl = p2.tile([1, R], F32, tag="rl", name=f"rl{blk}_{h}")
                        nc.vector.reciprocal(rl[:], avps[HD:HD + 1, :R])
                        rlb = psA.tile([128, 512], F32, tag="psA", name=f"rlb{blk}_{h}")
                        nc.tensor.matmul(rlb[:HD, :R], ones_row[:1, :HD], rl[:1, :],
                                         start=True, stop=True)
                        av_sb = p2.tile([HD, R], F32, tag="av_sb", name=f"avs{blk}_{h}")
                        nc.vector.tensor_copy(av_sb[:], avps[0:HD, :R])
                        nc.vector.tensor_mul(attnT[off:off + HD, dt, :],
                                             av_sb[:], rlb[:HD, :R])
                return attnT

            # ---------------- self-attention block ----------------
            xn = rmsnorm(xt_sb, "xn")
            wk_sb = load_w(W["sa_wk"], name="sa_wk_sb")
            wv_sb = load_w(W["sa_wv"], name="sa_wv_sb")
            k1, v1 = kv_block(wk_sb, wv_sb, xn, 0)

            wq_sb = load_w(W["sa_wq"], name="sa_wq_sb")
            qT = p1.tile([128, HT, R], BF16, tag="qt", name="qT")
            proj(wq_sb, xn, qT)

            attnT = attention(qT, k1, v1, 0)

            # cross-attn K/V depend only on raw context; their AGs overlap the
            # tail of self-attention instead of gating its start (collectives
            # execute strictly in stream order).
            wk2_sb = load_w(W["ca_wk"], name="ca_wk_sb")
            wv2_sb = load_w(W["ca_wv"], name="ca_wv_sb")
            k2, v2 = kv_block(wk2_sb, wv2_sb, ctx_sb, 1)
            wo_sb = load_w(W["sa_wo"], name="sa_wo_sb")
            h1 = presid.tile([128, HT, R], F32, tag="resid", name="h1")
            proj_add(wo_sb, attnT, xt_sb, h1)

            # ---------------- cross-attention block ----------------
            hn = rmsnorm(h1, "hn")
            wq2_sb = load_w(W["ca_wq"], name="ca_wq_sb")
            qT2 = p1.tile([128, HT, R], BF16, tag="qt", name="qT2")
            proj(wq2_sb, hn, qT2)

            attnT2 = attention(qT2, k2, v2, 1)
            wo2_sb = load_w(W["ca_wo"], name="ca_wo_sb")
            h2 = presid.tile([128, HT, R], F32, tag="resid", name="h2")
            proj_add(wo2_sb, attnT2, h1, h2)

            # ---------------- MLP block ----------------
            # NOTE: start=True clears has_written for the WHOLE psum bank, so each
            # accumulation group must own its bank exclusively for its entire
            # lifetime.  Phase A computes all 32 act subtiles into SBUF; phase B
            # runs one contiguous 32-matmul accumulation per output tile.
            hn2 = rmsnorm(h2, "hn2")
            NCHUNK = 4  # I-chunks of 1024
            act_full = p1.tile([128, I // 128, R], BF16, tag="act_full")  # 2MB
            wds = []
            for c in range(NCHUNK):
                wg_sb = load_w(W["w_gate"], cols=(1024 * c, 1024 * (c + 1)), name=f"wg{c}")
                wu_sb = load_w(W["w_up"], cols=(1024 * c, 1024 * (c + 1)), name=f"wu{c}")
                for mi in range(8):
                    gps = psA.tile([128, 512], F32, tag="psA", name=f"g{c}_{mi}")
                    for t in range(HT):
                        nc.tensor.matmul(gps[:, :R], wg_sb[:, t, 128 * mi:128 * (mi + 1)],
                                         hn2[:, t, :], start=(t == 0), stop=(t == HT - 1))
                    ups = psA.tile([128, 512], F32, tag="psA", name=f"u{c}_{mi}")
                    for t in range(HT):
                        nc.tensor.matmul(ups[:, :R], wu_sb[:, t, 128 * mi:128 * (mi + 1)],
                                         hn2[:, t, :], start=(t == 0), stop=(t == HT - 1))
                    gsil = p2.tile([128, R], BF16, tag="gsil", name=f"gs{c}_{mi}")
                    nc.scalar.activation(gsil[:], gps[:, :R], AF.Silu)
                    nc.vector.tensor_mul(act_full[:, 8 * c + mi, :], ups[:, :R], gsil[:])
            for c in range(NCHUNK):
                wds.append(load_w(W["w_down"], rows=(1024 * c, 1024 * (c + 1)), name=f"wd{c}"))
            out_sb = p1.tile([128, HT, R], F32, tag="out_sb")
            for m in range(HT):
                dps = psB.tile([128, 512], F32, tag="psB", name=f"dp{m}")
                for s in range(I // 128):
                    nc.tensor.matmul(dps[:, :R], wds[s // 8][:, s % 8, 128 * m:128 * (m + 1)],
                                     act_full[:, s, :],
                                     start=(s == 0), stop=(s == I // 128 - 1))
                nc.vector.tensor_add(out_sb[:, m, :], dps[:, :R], h2[:, m, :])
            nc.sync.dma_start(outT.rearrange("(t p) q -> p t q", p=128), out_sb[:])

    _split_multi_waits(nc)
    _CACHED_MODULE = nc
    return nc


def prep_in_maps(hidden_states, context, sa_norm_w, sa_wq, sa_wk, sa_wv, sa_wo,
                 ca_norm_w, ca_wq, ca_wk, ca_wv, ca_wo,
                 mlp_norm_w, w_gate, w_up, w_down):
    f32 = np.float32
    x = np.asarray(hidden_states, f32).reshape(S, H)
    ctx = np.asarray(context, f32).reshape(S, H)
    xT_full = np.ascontiguousarray(x.T)                      # [H, S] f32
    ctxT_full = np.ascontiguousarray(ctx.T).astype(BF16NP)   # [H, S] bf16

    def bf(a):
        return np.ascontiguousarray(np.asarray(a, f32)).astype(BF16NP)

    sa_w = np.asarray(sa_norm_w, f32)[:, None]
    ca_w = np.asarray(ca_norm_w, f32)[:, None]
    mlp_w = np.asarray(mlp_norm_w, f32)[:, None]
    scale = HD ** -0.5
    shared = {
        "sa_wq": bf(sa_w * np.asarray(sa_wq, f32) * scale),
        "sa_wk": bf(sa_w * np.asarray(sa_wk, f32)),
        "sa_wv": bf(sa_w * np.asarray(sa_wv, f32)),
        "sa_wo": bf(sa_wo),
        "ca_wq": bf(ca_w * np.asarray(ca_wq, f32) * scale),
        "ca_wk": bf(ca_wk),
        "ca_wv": bf(ca_wv),
        "ca_wo": bf(ca_wo),
        "w_gate": bf(mlp_w * np.asarray(w_gate, f32)),
        "w_up": bf(mlp_w * np.asarray(w_up, f32)),
        "w_down": bf(w_down),
    }
    in_maps = []
    for r in range(NC):
        m = dict(shared)
        m["xT"] = np.ascontiguousarray(xT_full[:, r * R:(r + 1) * R])
        m["ctxT"] = np.ascontiguousarray(ctxT_full[:, r * R:(r + 1) * R])
        in_maps.append(m)
    return in_maps


def run_spmd(in_maps, **kwargs):
    from concourse.bass_utils import run_bass_kernel_spmd
    nc = build_module()
    return run_bass_kernel_spmd(nc, in_maps, core_ids=list(range(NC)), **kwargs)


def kernel(**inputs):
    in_maps = prep_in_maps(**inputs)
    res = run_spmd(in_maps)
    out = np.empty((1, S, H), np.float32)
    for r in range(NC):
        out[0, r * R:(r + 1) * R, :] = res.results[r]["outT"].T
    return out


# revision 16
# speedup vs baseline: 1.0845x; 1.0845x over previous
"""Trainium2 Bass kernel for nn_DFlashDecoderLayer (dense transformer decoder layer:
self-attn + cross-attn + SwiGLU MLP, B=1, S=2048, H=1024, NH=16, HD=64, I=4096).

Sharding strategy (8 NeuronCores, SPMD):
  Sequence-sharded: core r owns query rows [256r, 256r+256).  Every matmul weight is
  used in full by every core, in bf16.  The only cross-core data dependency is full-
  sequence K/V for the two attention blocks; each core computes K/V for its own rows
  (all heads) and a single AllGather per attention block shares them (1MB/rank).
  Norm weights + 1/sqrt(HD) score scale are folded into the following projection
  weights host-side.  Residual path stays fp32; matmuls run bf16 with fp32 PSUM
  accumulation; softmax exp in fp32->bf16 without max-subtraction (|scores| < ~5).

  On-chip layout is feature-major ("transposed"): activations live as [H_part, seq]
  so weight matrices ([in, out] row-major) serve directly as matmul lhsT tiles and
  layer outputs chain without transposes.  Scores are computed transposed [k, q];
  the softmax denominator comes from a ones-column augmented onto V (M=65 matmul),
  and the 1/l division is applied via gpsimd partition-broadcast before o_proj.
"""

import os
import sys

sys.path.insert(0, "/opt/trn_rl_repo")

import numpy as np
import ml_dtypes

import concourse.bass as bass
import concourse.mybir as mybir
import concourse.tile as tile

H = 1024      # hidden size
S = 2048      # sequence length
NH = 16       # heads
HD = 64       # head dim
I = 4096      # mlp intermediate
NC = 8        # cores
R = S // NC   # rows per core = 256
HT = H // 128  # hidden tiles = 8
KT = S // 128  # key tiles = 16
EPS = 1e-6

F32 = mybir.dt.float32
BF16 = mybir.dt.bfloat16
AF = mybir.ActivationFunctionType
BF16NP = ml_dtypes.bfloat16

_CACHED_MODULE = None


def _split_multi_waits(nc):
    """This env's walrus rejects >1 sem wait per instruction.
    Hoist extra waits onto preceding single-wait NoOps on the same engine."""
    limit = 1
    n_split = 0
    for f in nc.m.functions:
        for bb in f.blocks:
            new_insts = []
            for inst in bb.instructions:
                si = getattr(inst, "sync_info", None)
                if (si is not None and getattr(inst, "engine", None) is not None
                        and len(si.on_wait) > limit):
                    waits = list(si.on_wait)
                    hoist, keep = waits[:-limit], waits[-limit:]
                    for i, w in enumerate(hoist):
                        new_insts.append(
                            mybir.InstNoOp(
                                name=f"{inst.name}_waitsplit_{i}",
                                engine=inst.engine,
                                sync_info=mybir.SyncInfo(on_wait=[w], on_update=[]),
                                bass_nofuse=True,
                            )
                        )
                        n_split += 1
                    si.on_wait = keep
                new_insts.append(inst)
            bb.instructions = new_insts
    return n_split


def build_module():
    global _CACHED_MODULE
    if _CACHED_MODULE is not None:
        return _CACHED_MODULE

    nc = bass.Bass(num_devices=NC)

    # --- kernel I/O (per-core) ---
    xT = nc.declare_dram_parameter("xT", [H, R], F32, isOutput=False)
    ctxT = nc.declare_dram_parameter("ctxT", [H, R], BF16, isOutput=False)
    wnames = ["sa_wq", "sa_wk", "sa_wv", "sa_wo", "ca_wq", "ca_wk", "ca_wv", "ca_wo"]
    W = {n: nc.declare_dram_parameter(n, [H, H], BF16, isOutput=False) for n in wnames}
    W["w_gate"] = nc.declare_dram_parameter("w_gate", [H, I], BF16, isOutput=False)
    W["w_up"] = nc.declare_dram_parameter("w_up", [H, I], BF16, isOutput=False)
    W["w_down"] = nc.declare_dram_parameter("w_down", [I, H], BF16, isOutput=False)
    outT = nc.declare_dram_parameter("outT", [H, R], F32, isOutput=True)

    groups = [list(range(NC))]

    with tile.TileContext(nc) as tc:
        with (
            tc.tile_pool(name="p1", bufs=1) as p1,        # long-lived singles
            tc.tile_pool(name="p2", bufs=2) as p2,        # rotating pairs
            tc.tile_pool(name="resid", bufs=2) as presid, # xT / h1 / h2 fp32
            tc.tile_pool(name="wts", bufs=4) as pw,       # 2MB weight slabs
            tc.tile_pool(name="psA", bufs=4, space="PSUM") as psA,
            tc.tile_pool(name="psB", bufs=4, space="PSUM") as psB,
            tc.tile_pool(name="dram", bufs=1, space="DRAM") as pdram,
        ):
            # --- constants ---
            inv_h = p1.tile([128, 1], F32, tag="inv_h")
            nc.vector.memset(inv_h[:], 1.0 / H)
            eps_c = p1.tile([1, 1], F32, tag="eps_c")
            nc.vector.memset(eps_c[:], EPS)
            # ones row for K=1 outer-product broadcasts (partition_broadcast is
            # unsupported by this walrus build)
            ones_row = p1.tile([1, 128], F32, tag="ones_row")
            nc.vector.memset(ones_row[:], 1.0)

            # --- input loads ---
            xt_sb = presid.tile([128, HT, R], F32, tag="resid", name="xt_sb")
            nc.sync.dma_start(xt_sb[:], xT.rearrange("(t p) q -> p t q", p=128))
            ctx_sb = p1.tile([128, HT, R], BF16, tag="ctx_sb")
            nc.sync.dma_start(ctx_sb[:], ctxT.rearrange("(t p) q -> p t q", p=128))

            def load_w(dram_t, cols=None, rows=None, name="w"):
                """Load a [1024, M<=1024] slab of a weight matrix as [128, 8, M] bf16."""
                ap = dram_t.rearrange("(t p) m -> p t m", p=128)
                if rows is not None:  # row-chunk of a tall matrix (w_down)
                    ap = dram_t[rows[0]:rows[1], :].rearrange("(t p) m -> p t m", p=128)
                if cols is not None:
                    ap = ap[:, :, cols[0]:cols[1]]
                m = ap.shape[2]
                t = pw.tile([128, HT, 1024], BF16, tag="w", name=name)
                nc.sync.dma_start(t[:, :, :m], ap)
                return t

            def rmsnorm(src_f32, dst_name):
                """src [128, HT, R] f32 -> normalized bf16 [128, HT, R] (no weight)."""
                var = psA.tile([128, 512], F32, tag="psA", name=f"{dst_name}_var")
                for t in range(HT):
                    sq = p2.tile([128, R], F32, tag="sq", name=f"{dst_name}_sq{t}")
                    nc.scalar.activation(sq[:], src_f32[:, t, :], AF.Square)
                    nc.tensor.matmul(var[:1, :R], inv_h[:], sq[:],
                                     start=(t == 0), stop=(t == HT - 1))
                sd = p2.tile([1, R], F32, tag="sd", name=f"{dst_name}_sd")
                nc.scalar.activation(sd[:], var[:1, :R], AF.Sqrt, bias=eps_c[:])
                rstd = p2.tile([1, R], F32, tag="rstd", name=f"{dst_name}_rstd")
                nc.vector.reciprocal(rstd[:], sd[:])
                rb = psA.tile([128, 512], F32, tag="psA", name=f"{dst_name}_rb")
                nc.tensor.matmul(rb[:, :R], ones_row[:1, :], rstd[:1, :],
                                 start=True, stop=True)
                dst = p1.tile([128, HT, R], BF16, tag="normed", name=dst_name)
                for t in range(HT):
                    nc.vector.tensor_mul(dst[:, t, :], src_f32[:, t, :], rb[:, :R])
                return dst

            def proj(w_sb, act_sb, dst_bf16):
                """dst[*, m, :] (bf16 [128, HT, R]) = (W.T @ act), W slab [128, HT, 1024]."""
                for m in range(HT):
                    ps = psA.tile([128, 512], F32, tag="psA", name=f"pj_{m}")
                    for t in range(HT):
                        nc.tensor.matmul(ps[:, :R], w_sb[:, t, 128 * m:128 * (m + 1)],
                                         act_sb[:, t, :], start=(t == 0), stop=(t == HT - 1))
                    nc.vector.tensor_copy(dst_bf16[:, m, :], ps[:, :R])

            def proj_add(w_sb, act_sb, resid_f32, dst_f32):
                """dst (f32 [128, HT, R]) = resid + W.T @ act."""
                for m in range(HT):
                    ps = psA.tile([128, 512], F32, tag="psA", name=f"pa_{m}")
                    for t in range(HT):
                        nc.tensor.matmul(ps[:, :R], w_sb[:, t, 128 * m:128 * (m + 1)],
                                         act_sb[:, t, :], start=(t == 0), stop=(t == HT - 1))
                    nc.vector.tensor_add(dst_f32[:, m, :], ps[:, :R], resid_f32[:, m, :])

            def kv_block(wk_sb, wv_sb, act_sb, blk):
                """Compute own-row K^T [1024, R] and V [R, 1024] (bf16), AllGather
                each across cores (K first — scores only need K, so attention can
                start while the V gather is still in flight)."""
                k_in = pdram.tile([H * R], BF16, tag=f"kin{blk}")
                k_out = pdram.tile([NC, H * R], BF16, tag=f"kout{blk}",
                                   addr_space="Shared")
                v_in = pdram.tile([H * R], BF16, tag=f"vin{blk}")
                v_out = pdram.tile([NC, H * R], BF16, tag=f"vout{blk}",
                                   addr_space="Shared")
                k_view = k_in.rearrange("(t p q) -> p t q", t=HT, p=128, q=R)
                v_view = v_in.rearrange("(mt p d) -> p mt d", mt=2, p=128, d=1024)
                for m in range(HT):
                    ps = psA.tile([128, 512], F32, tag="psA", name=f"k{blk}_{m}")
                    for t in range(HT):
                        nc.tensor.matmul(ps[:, :R], wk_sb[:, t, 128 * m:128 * (m + 1)],
                                         act_sb[:, t, :], start=(t == 0), stop=(t == HT - 1))
                    stg = p2.tile([128, 512], BF16, tag="stg", bufs=3, name=f"ks{blk}_{m}")
                    nc.vector.tensor_copy(stg[:, :R], ps[:, :R])
                    nc.sync.dma_start(k_view[:, m, :], stg[:, :R])
                nc.gpsimd.collective_compute(
                    "AllGather", mybir.AluOpType.bypass, replica_groups=groups,
                    ins=[k_in[:]], outs=[k_out[:]])
                for mt in range(2):
                    for nchunk in range(2):
                        ps = psA.tile([128, 512], F32, tag="psA", name=f"v{blk}_{mt}_{nchunk}")
                        for t in range(HT):
                            nc.tensor.matmul(
                                ps[:], act_sb[:, t, 128 * mt:128 * (mt + 1)],
                                wv_sb[:, t, 512 * nchunk:512 * (nchunk + 1)],
                                start=(t == 0), stop=(t == HT - 1))
                        stg = p2.tile([128, 512], BF16, tag="stg", bufs=3,
                                      name=f"vs{blk}_{mt}_{nchunk}")
                        nc.vector.tensor_copy(stg[:], ps[:])
                        nc.sync.dma_start(v_view[:, mt, 512 * nchunk:512 * (nchunk + 1)], stg[:])
                nc.gpsimd.collective_compute(
                    "AllGather", mybir.AluOpType.bypass, replica_groups=groups,
                    ins=[v_in[:]], outs=[v_out[:]])
                return k_out, v_out

            def attention(q_sb, k_out, v_out, blk):
                """q_sb [128, HT, R] bf16 (feature-major, all heads), k_out/v_out from
                kv_block. Returns attnT [128, HT, R] bf16 = softmax(qk)V transposed."""
                vsb = p1.tile([128, KT, NH, HD + 1], BF16, tag="vsb", name=f"vsb{blk}")
                attnT = p1.tile([128, HT, R], BF16, tag="attnT", name=f"attnT{blk}")
                for dt in range(HT):  # head pair dt = heads 2dt, 2dt+1
                    kf = p2.tile([128, NC, R], BF16, tag="kf", name=f"kf{blk}_{dt}")
                    for r in range(NC):
                        nc.sync.dma_start(
                            kf[:, r, :],
                            k_out[r].rearrange("(t p q) -> p t q", t=HT, p=128, q=R)[:, dt, :])
                    if dt == 0:
                        # V table loads queue AFTER the first K loads so head-0
                        # scores aren't stuck behind 4MB of V DMA.
                        for kt in range(KT):
                            r, mt = kt // 2, kt % 2
                            src = v_out[r].rearrange(
                                "(mt p hd d) -> p mt hd d", mt=2, p=128, hd=NH, d=HD)
                            nc.sync.dma_start(vsb[:, kt, :, 0:HD], src[:, mt, :, :])
                        nc.vector.memset(vsb[:, :, :, HD:HD + 1], 1.0)
                    for hh in range(2):
                        h = 2 * dt + hh
                        off = HD * hh
                        pt = p2.tile([128, KT, R], BF16, tag="pt", bufs=3, name=f"pt{blk}_{h}")
                        for kp in range(KT // 2):
                            sps = psA.tile([128, 512], F32, tag="psA", name=f"s{blk}_{h}_{kp}")
                            for j in range(2):
                                kt = 2 * kp + j
                                r2, mt2 = kt // 2, kt % 2
                                nc.tensor.matmul(
                                    sps[:, R * j:R * (j + 1)],
                                    kf[off:off + HD, r2, 128 * mt2:128 * (mt2 + 1)],
                                    q_sb[off:off + HD, dt, :],
                                    start=True, stop=True, tile_position=(off, 0))
                            nc.scalar.activation(
                                pt[:, 2 * kp:2 * kp + 2, :],
                                sps.rearrange("p (a b) -> p a b", a=2), AF.Exp)
                        avps = psB.tile([128, 512], F32, tag="psB", name=f"av{blk}_{h}")
                        for kt in range(KT):
                            nc.tensor.matmul(avps[:HD + 1, :R], vsb[:, kt, h, :],
                                             pt[:, kt, :], start=(kt == 0), stop=(kt == KT - 1))
                        rl = p2.tile([1, R], F32, tag="rl", name=f"rl{blk}_{h}")
                        nc.vector.reciprocal(rl[:], avps[HD:HD + 1, :R])
                        rlb = psA.tile([128, 512], F32, tag="psA", name=f"rlb{blk}_{h}")
                        nc.tensor.matmul(rlb[:HD, :R], ones_row[:1, :HD], rl[:1, :],
                                         start=True, stop=True)
                        av_sb = p2.tile([HD, R], F32, tag="av_sb", name=f"avs{blk}_{h}")
                        nc.vector.tensor_copy(av_sb[:], avps[0:HD, :R])
                        nc.vector.tensor_mul(attnT[off:off + HD, dt, :],
                                             av_sb[:], rlb[:HD, :R])
                return attnT

            # ---------------- self-attention block ----------------
            xn = rmsnorm(xt_sb, "xn")
            wk_sb = load_w(W["sa_wk"], name="sa_wk_sb")
            wv_sb = load_w(W["sa_wv"], name="sa_wv_sb")
            k1, v1 = kv_block(wk_sb, wv_sb, xn, 0)

            # cross-attn K/V depend only on raw context: compute + AG them early
            # so both gathers overlap the self-attention epilogue projections.
            wk2_sb = load_w(W["ca_wk"], name="ca_wk_sb")
            wv2_sb = load_w(W["ca_wv"], name="ca_wv_sb")
            k2, v2 = kv_block(wk2_sb, wv2_sb, ctx_sb, 1)

            wq_sb = load_w(W["sa_wq"], name="sa_wq_sb")
            qT = p1.tile([128, HT, R], BF16, tag="qt", name="qT")
            proj(wq_sb, xn, qT)

            attnT = attention(qT, k1, v1, 0)
            wo_sb = load_w(W["sa_wo"], name="sa_wo_sb")
            h1 = presid.tile([128, HT, R], F32, tag="resid", name="h1")
            proj_add(wo_sb, attnT, xt_sb, h1)

            # ---------------- cross-attention block ----------------
            hn = rmsnorm(h1, "hn")
            wq2_sb = load_w(W["ca_wq"], name="ca_wq_sb")
            qT2 = p1.tile([128, HT, R], BF16, tag="qt", name="qT2")
            proj(wq2_sb, hn, qT2)

            attnT2 = attention(qT2, k2, v2, 1)
            wo2_sb = load_w(W["ca_wo"], name="ca_wo_sb")
            h2 = presid.tile([128, HT, R], F32, tag="resid", name="h2")
            proj_add(wo2_sb, attnT2, h1, h2)

            # ---------------- MLP block ----------------
            # NOTE: start=True clears has_written for the WHOLE psum bank, so each
            # accumulation group must own its bank exclusively for its entire
            # lifetime.  Phase A computes all 32 act subtiles into SBUF; phase B
            # runs one contiguous 32-matmul accumulation per output tile.
            hn2 = rmsnorm(h2, "hn2")
            NCHUNK = 4  # I-chunks of 1024
            act_full = p1.tile([128, I // 128, R], BF16, tag="act_full")  # 2MB
            wds = []
            for c in range(NCHUNK):
                wg_sb = load_w(W["w_gate"], cols=(1024 * c, 1024 * (c + 1)), name=f"wg{c}")
                wu_sb = load_w(W["w_up"], cols=(1024 * c, 1024 * (c + 1)), name=f"wu{c}")
                for mi in range(8):
                    gps = psA.tile([128, 512], F32, tag="psA", name=f"g{c}_{mi}")
                    for t in range(HT):
                        nc.tensor.matmul(gps[:, :R], wg_sb[:, t, 128 * mi:128 * (mi + 1)],
                                         hn2[:, t, :], start=(t == 0), stop=(t == HT - 1))
                    ups = psA.tile([128, 512], F32, tag="psA", name=f"u{c}_{mi}")
                    for t in range(HT):
                        nc.tensor.matmul(ups[:, :R], wu_sb[:, t, 128 * mi:128 * (mi + 1)],
                                         hn2[:, t, :], start=(t == 0), stop=(t == HT - 1))
                    gsil = p2.tile([128, R], BF16, tag="gsil", name=f"gs{c}_{mi}")
                    nc.scalar.activation(gsil[:], gps[:, :R], AF.Silu)
                    nc.vector.tensor_mul(act_full[:, 8 * c + mi, :], ups[:, :R], gsil[:])
            for c in range(NCHUNK):
                wds.append(load_w(W["w_down"], rows=(1024 * c, 1024 * (c + 1)), name=f"wd{c}"))
            out_sb = p1.tile([128, HT, R], F32, tag="out_sb")
            for m in range(HT):
                dps = psB.tile([128, 512], F32, tag="psB", name=f"dp{m}")
                for s in range(I // 128):
                    nc.tensor.matmul(dps[:, :R], wds[s // 8][:, s % 8, 128 * m:128 * (m + 1)],
                                     act_full[:, s, :],
                                     start=(s == 0), stop=(s == I // 128 - 1))
                nc.vector.tensor_add(out_sb[:, m, :], dps[:, :R], h2[:, m, :])
            nc.sync.dma_start(outT.rearrange("(t p) q -> p t q", p=128), out_sb[:])

    _split_multi_waits(nc)
    _CACHED_MODULE = nc
    return nc


def prep_in_maps(hidden_states, context, sa_norm_w, sa_wq, sa_wk, sa_wv, sa_wo,
                 ca_norm_w, ca_wq, ca_wk, ca_wv, ca_wo,
                 mlp_norm_w, w_gate, w_up, w_down):
    f32 = np.float32
    x = np.asarray(hidden_states, f32).reshape(S, H)
    ctx = np.asarray(context, f32).reshape(S, H)
    xT_full = np.ascontiguousarray(x.T)                      # [H, S] f32
    ctxT_full = np.ascontiguousarray(ctx.T).astype(BF16NP)   # [H, S] bf16

    def bf(a):
        return np.ascontiguousarray(np.asarray(a, f32)).astype(BF16NP)

    sa_w = np.asarray(sa_norm_w, f32)[:, None]
    ca_w = np.asarray(ca_norm_w, f32)[:, None]
    mlp_w = np.asarray(mlp_norm_w, f32)[:, None]
    scale = HD ** -0.5
    shared = {
        "sa_wq": bf(sa_w * np.asarray(sa_wq, f32) * scale),
        "sa_wk": bf(sa_w * np.asarray(sa_wk, f32)),
        "sa_wv": bf(sa_w * np.asarray(sa_wv, f32)),
        "sa_wo": bf(sa_wo),
        "ca_wq": bf(ca_w * np.asarray(ca_wq, f32) * scale),
        "ca_wk": bf(ca_wk),
        "ca_wv": bf(ca_wv),
        "ca_wo": bf(ca_wo),
        "w_gate": bf(mlp_w * np.asarray(w_gate, f32)),
        "w_up": bf(mlp_w * np.asarray(w_up, f32)),
        "w_down": bf(w_down),
    }
    in_maps = []
    for r in range(NC):
        m = dict(shared)
        m["xT"] = np.ascontiguousarray(xT_full[:, r * R:(r + 1) * R])
        m["ctxT"] = np.ascontiguousarray(ctxT_full[:, r * R:(r + 1) * R])
        in_maps.append(m)
    return in_maps


def run_spmd(in_maps, **kwargs):
    from concourse.bass_utils import run_bass_kernel_spmd
    nc = build_module()
    return run_bass_kernel_spmd(nc, in_maps, core_ids=list(range(NC)), **kwargs)


def kernel(**inputs):
    in_maps = prep_in_maps(**inputs)
    res = run_spmd(in_maps)
    out = np.empty((1, S, H), np.float32)
    for r in range(NC):
        out[0, r * R:(r + 1) * R, :] = res.results[r]["outT"].T
    return out
